# revision 12
# baseline (speedup 1.0000x reference)
"""Trainium2 Bass kernel for the trilinear scatter -> conv3d x3 -> gather module.

Algorithm (per core, data-parallel over points):
  Phase 1 (scatter): for each 2048-point slab, build the trilinear scatter
    matrix S [128, 512] per 128-point group via separable periodic hat
    functions (relu(||iota-8p|-4|-3), all on ScalarE), outer-product them in
    bf16 (DVE 2x mode), and accumulate img^T[32f, 512cells] with PE matmuls
    alternating two PSUM banks. Base cell ids stay in SBUF (int32) for the
    gpsimd gather path and go to DRAM as an fp16 row (transposed to
    point-major g*128+p order) for the PE gather path.
  Phase 2: AllReduce img^T across the 8 cores (64KB).
  Phase 3 (redundant on all cores): 3 periodic 3x3x3 convs in bf16 as 27
    PSUM-accumulated matmuls each over a wrap-padded [32, 10^3] lattice,
    with silu residual; box-filter -> gather table T[512, 32] (f32 in DRAM
    for the indirect path, bf16 chunks in SBUF for the PE path).
  Phase 4 (hybrid gather): ~30% of slabs gather T rows via gpsimd indirect
    DMA; the rest run a PE one-hot path: broadcast the fp16 cell-id row to
    128 partitions (HWDGE), is_equal vs per-partition cell iota (bf16
    one-hot), 4 matmuls vs T^T chunks into PSUM, PE-transpose back to
    point-major, assemble on ScalarE.
"""
import os
import sys

for _p in ("/root/.axon_site/_ro/trn_rl_repo", "/opt/trn_rl_repo"):
    if _p not in sys.path and os.path.isdir(_p):
        sys.path.append(_p)

import numpy as np
from contextlib import ExitStack

import bass_rust
import concourse.bass as bass
import concourse.tile as tile
from concourse import mybir
from concourse.bass_utils import run_bass_kernel_spmd
from concourse.masks import make_identity

F32 = mybir.dt.float32
BF16 = mybir.dt.bfloat16
F16 = mybir.dt.float16
I32 = mybir.dt.int32
AF = mybir.ActivationFunctionType
OP = mybir.AluOpType

NCORES = 8
P = 128
G = 16                  # point-groups per slab (points per partition)
SLAB = P * G            # 2048 points per slab
RES = 8
NCELL = RES ** 3        # 512
CH = 32                 # cin = chid = cout = 32


def _legalize_sync_waits(nc, max_waits=1):
    """The walrus codegen on this path rejects instructions with more than
    ~1 extra sync wait; split excess waits onto preceding same-engine NoOps."""
    ctr = 0
    for f in nc.m.functions:
        for bb in f.blocks:
            insts = list(bb.instructions)
            new_insts = []
            changed = False
            for inst in insts:
                si = inst.sync_info
                if si is not None and len(si.on_wait) > max_waits:
                    waits = list(si.on_wait)
                    excess, keep = waits[:-max_waits], waits[-max_waits:]
                    for c0 in range(0, len(excess), max_waits):
                        chunk = excess[c0:c0 + max_waits]
                        nop = mybir.InstNoOp(name=f"lgw-{ctr}", ins=[], outs=[])
                        ctr += 1
                        nop.engine = inst.engine
                        nop.sync_info = bass_rust.SyncInfo(on_wait=chunk, on_update=[])
                        new_insts.append(nop)
                    si.on_wait = keep
                    changed = True
                new_insts.append(inst)
            if changed:
                bb.instructions = new_insts
    return nc


def _pad3_views(dst_pad, src, n=RES, pad_lo=1, pad_hi=1):
    """Yield (dst_view, src_view) block pairs to fill a wrap-padded lattice.

    dst_pad: AP [CH, (n+pad_lo+pad_hi)^3] ; src: AP [CH, n^3].
    """
    m = n + pad_lo + pad_hi
    dv = dst_pad.rearrange("p (x y z) -> p x y z", y=m, z=m)
    sv = src.rearrange("p (x y z) -> p x y z", y=n, z=n)
    segs = []
    if pad_lo:
        segs.append((0, n - pad_lo, pad_lo))
    segs.append((pad_lo, 0, n))
    if pad_hi:
        segs.append((pad_lo + n, 0, pad_hi))
    for dx, sx, lx in segs:
        for dy, sy, ly in segs:
            for dz, sz, lz in segs:
                yield (
                    dv[:, dx:dx + lx, dy:dy + ly, dz:dz + lz],
                    sv[:, sx:sx + lx, sy:sy + ly, sz:sz + lz],
                )


def _conv_layer(nc, psum_out, pad_t, w_t, m=10):
    """27 PSUM-accumulated matmuls: out[32co, 512] += K_o^T @ pad[32ci, shifted]."""
    pv = pad_t.rearrange("p (x y z) -> p x y z", y=m, z=m)
    o = 0
    for kx in range(3):
        for ky in range(3):
            for kz in range(3):
                rhs = pv[:, kx:kx + RES, ky:ky + RES, kz:kz + RES]
                nc.tensor.matmul(
                    out=psum_out,
                    lhsT=w_t[:, 32 * o:32 * (o + 1)],
                    rhs=rhs,
                    start=(o == 0),
                    stop=(o == 26),
                )
                o += 1


def build_nc(n_slabs):
    nrows = n_slabs * P          # rows in slab-layout [nrows, G*d]
    n_gp = max(0, min(n_slabs, round(n_slabs * 8 / 31)))  # gpsimd-path slabs
    gp_set = set(range(n_slabs - n_gp, n_slabs))
    nc = bass.Bass("TRN2", target_bir_lowering=False, debug=False,
                   num_devices=NCORES)

    pos_ext = nc.dram_tensor("pos", [nrows, 3 * G], F32, kind="ExternalInput").ap()
    feat_ext = nc.dram_tensor("feat", [nrows, CH * G], F32, kind="ExternalInput").ap()
    wenc_ext = nc.dram_tensor("wenc", [CH, 27 * CH], F32, kind="ExternalInput").ap()
    winner_ext = nc.dram_tensor("winner", [CH, 27 * CH], F32, kind="ExternalInput").ap()
    wdec_ext = nc.dram_tensor("wdec", [CH, 27 * CH], F32, kind="ExternalInput").ap()
    out_ext = nc.dram_tensor("out", [nrows, CH * G], F32, kind="ExternalOutput").ap()

    t_dram = nc.dram_tensor("t_table", [NCELL, CH], F32).ap()
    brow_dram = nc.dram_tensor("brow", [max(n_slabs, 1), SLAB], F16).ap()
    ar_in = nc.dram_tensor("ar_in", [CH, NCELL], F32).ap()
    ar_out = nc.dram_tensor("ar_out", [CH, NCELL], F32).ap()

    with tile.TileContext(nc) as tc, ExitStack() as ctx:
        const_pool = ctx.enter_context(tc.tile_pool(name="const", bufs=1))
        in_pool = ctx.enter_context(tc.tile_pool(name="in", bufs=3))
        work_pool = ctx.enter_context(tc.tile_pool(name="work", bufs=3))
        s_pool = ctx.enter_context(tc.tile_pool(name="s", bufs=4))
        lat_pool = ctx.enter_context(tc.tile_pool(name="lat", bufs=1))
        g_pool = ctx.enter_context(tc.tile_pool(name="g", bufs=4))
        bi_pool = ctx.enter_context(tc.tile_pool(name="bi", bufs=1))
        eq_pool = ctx.enter_context(tc.tile_pool(name="eq", bufs=3))
        ot_pool = ctx.enter_context(tc.tile_pool(name="ot", bufs=4))
        psum_img = ctx.enter_context(tc.tile_pool(name="pimg", bufs=1, space="PSUM"))
        psum_conv = ctx.enter_context(tc.tile_pool(name="pconv", bufs=1, space="PSUM"))
        psum_tr = ctx.enter_context(tc.tile_pool(name="ptr", bufs=2, space="PSUM"))
        psum_bft = ctx.enter_context(tc.tile_pool(name="pbft", bufs=1, space="PSUM"))
        psum_ot = ctx.enter_context(tc.tile_pool(name="pot", bufs=2, space="PSUM"))

        # constants
        iota_i = const_pool.tile([P, RES], I32)
        nc.gpsimd.iota(iota_i[:], pattern=[[1, RES]], base=0, channel_multiplier=0)
        iota_f = const_pool.tile([P, RES], F32)
        nc.scalar.copy(iota_f[:], iota_i[:])
        bias_m4 = const_pool.tile([P, 1], F32)
        nc.gpsimd.memset(bias_m4[:], -4.0)
        bias_m3 = const_pool.tile([P, 1], F32)
        nc.gpsimd.memset(bias_m3[:], -3.0)
        ident32b = const_pool.tile([CH, CH], BF16)
        make_identity(nc, ident32b[:])
        ident128h = const_pool.tile([P, P], F16)
        make_identity(nc, ident128h[:])
        # per-partition cell ids for the 4 one-hot chunks: c*128 + p (fp16)
        iota_cell = []
        for c in range(4):
            ii = const_pool.tile([P, 1], I32, tag=f"ioc{c}")
            nc.gpsimd.iota(ii[:], pattern=[[0, 1]], base=c * P,
                           channel_multiplier=1)
            ih = const_pool.tile([P, 1], F16, tag=f"ioh{c}")
            nc.scalar.copy(ih[:], ii[:])
            iota_cell.append(ih)

        iota_neg = []
        for c in range(4):
            iv = const_pool.tile([P, 1], F32, tag=f"ion{c}", name=f"ion{c}")
            nc.scalar.activation(iv[:], iota_cell[c][:], AF.Copy, scale=-1.0)
            iota_neg.append(iv)
        bias_p1 = const_pool.tile([P, 1], F32)
        nc.gpsimd.memset(bias_p1[:], 1.0)

        # weights
        wenc = const_pool.tile([CH, 27 * CH], F32)
        nc.sync.dma_start(wenc[:], wenc_ext[:])
        winner = const_pool.tile([CH, 27 * CH], F32)
        nc.sync.dma_start(winner[:], winner_ext[:])
        wdec = const_pool.tile([CH, 27 * CH], F32)
        nc.sync.dma_start(wdec[:], wdec_ext[:])
        wenc_b = const_pool.tile([CH, 27 * CH], BF16)
        nc.vector.tensor_copy(wenc_b[:], wenc[:])
        winner_b = const_pool.tile([CH, 27 * CH], BF16)
        nc.vector.tensor_copy(winner_b[:], winner[:])
        wdec_b = const_pool.tile([CH, 27 * CH], BF16)
        nc.vector.tensor_copy(wdec_b[:], wdec[:])

        img_a = psum_img.tile([CH, NCELL], F32, tag="a")
        img_b = psum_img.tile([CH, NCELL], F32, tag="b")

        bflat_i = {}
        # ---------------- Phase 1: scatter ----------------
        for s in range(n_slabs):
            rows = slice(s * P, (s + 1) * P)
            pos_t = in_pool.tile([P, 3 * G], F32, tag="pos")
            nc.sync.dma_start(pos_t[:], pos_ext[rows, :])
            feat_t = in_pool.tile([P, CH * G], F32, tag="feat")
            nc.scalar.dma_start(feat_t[:], feat_ext[rows, :])

            # sc = pos * 8
            sc = work_pool.tile([P, 3 * G], F32, tag="sc")
            nc.vector.tensor_scalar(out=sc[:], in0=pos_t[:], scalar1=float(RES),
                                    scalar2=None, op0=OP.mult)
            # exact floor: br = round(sc); bf = float(br); bff = bf - (bf > sc)
            br = work_pool.tile([P, 3 * G], I32, tag="br")
            nc.vector.tensor_copy(br[:], sc[:])
            bf = work_pool.tile([P, 3 * G], F32, tag="bf")
            nc.scalar.copy(bf[:], br[:])
            gt = work_pool.tile([P, 3 * G], F32, tag="gt")
            nc.vector.tensor_tensor(out=gt[:], in0=bf[:], in1=sc[:], op=OP.is_gt)
            bff = work_pool.tile([P, 3 * G], F32, tag="bff")
            nc.vector.tensor_tensor(out=bff[:], in0=bf[:], in1=gt[:], op=OP.subtract)

            # b_flat = bx*64 + by*8 + bz (exact in f32)
            bff3 = bff[:].rearrange("p (g d) -> p g d", d=3)
            t64 = work_pool.tile([P, G], F32, tag="t64")
            nc.vector.tensor_scalar(out=t64[:], in0=bff3[:, :, 0], scalar1=64.0,
                                    scalar2=None, op0=OP.mult)
            t8 = work_pool.tile([P, G], F32, tag="t8")
            nc.vector.tensor_scalar(out=t8[:], in0=bff3[:, :, 1], scalar1=8.0,
                                    scalar2=None, op0=OP.mult)
            bflat_f = work_pool.tile([P, G], F32, tag="bflat")
            nc.vector.tensor_tensor(out=bflat_f[:], in0=t64[:], in1=t8[:], op=OP.add)
            nc.vector.tensor_tensor(out=bflat_f[:], in0=bflat_f[:], in1=bff3[:, :, 2],
                                    op=OP.add)

            if s in gp_set:
                bfi = bi_pool.tile([P, G], I32, tag=f"bfi{s}")
                nc.vector.tensor_copy(bfi[:], bflat_f[:])
                bflat_i[s] = bfi
            else:
                # fp16 ids, transposed to point-major j = g*128+p, to DRAM
                bflat_h = work_pool.tile([P, G], F16, tag="bfh")
                nc.vector.tensor_copy(bflat_h[:], bflat_f[:])
                bfT_ps = psum_bft.tile([G, P], F16, tag="bfT")
                nc.tensor.transpose(out=bfT_ps[:], in_=bflat_h[:],
                                    identity=ident128h[:])
                bfT = work_pool.tile([G, P], F16, tag="bfTs")
                nc.scalar.copy(bfT[:], bfT_ps[:])
                nc.sync.dma_start(
                    brow_dram[s:s + 1, :].rearrange("r (p c) -> (r p) c", c=P),
                    bfT[:])

            # periodic hat weights, all dims at once: W[p, (g, d, 8)] bf16
            # ring-dist hat: W = relu(||iota - sc| - 4| - 3)
            E = work_pool.tile([P, G * 3 * RES], F32, tag="E")
            nc.vector.tensor_tensor(
                out=E[:].rearrange("p (g d e) -> p g d e", d=3, e=RES),
                in0=iota_f[:].unsqueeze(1).unsqueeze(1)
                    .to_broadcast([P, G, 3, RES]),
                in1=sc[:].rearrange("p (g d) -> p g d", d=3).unsqueeze(3)
                    .to_broadcast([P, G, 3, RES]),
                op=OP.subtract)
            A = work_pool.tile([P, G * 3 * RES], F32, tag="A")
            nc.scalar.activation(A[:], E[:], AF.Abs)
            B2 = work_pool.tile([P, G * 3 * RES], F32, tag="B2")
            nc.scalar.activation(B2[:], A[:], AF.Abs, bias=bias_m4[:], scale=1.0)
            W = work_pool.tile([P, G * 3 * RES], BF16, tag="W")
            nc.scalar.activation(W[:], B2[:], AF.Relu, bias=bias_m3[:], scale=1.0)
            Wv = W[:].rearrange("p (g d e) -> p g d e", d=3, e=RES)

            # T1[p, (g, y, z)] = WY[g,y] * WZ[g,z]  (bf16, 2x mode)
            T1 = work_pool.tile([P, G * 64], BF16, tag="T1")
            nc.vector.tensor_tensor(
                out=T1[:].rearrange("p (g y z) -> p g y z", y=RES, z=RES),
                in0=Wv[:, :, 1, :].unsqueeze(3).to_broadcast([P, G, RES, RES]),
                in1=Wv[:, :, 2, :].unsqueeze(2).to_broadcast([P, G, RES, RES]),
                op=OP.mult)

            # feat -> bf16 for PE
            feat_bf = work_pool.tile([P, CH * G], BF16, tag="featbf")
            nc.scalar.copy(feat_bf[:], feat_t[:])

            # S[p, (g, x, yz)] = WX[g,x] * T1[g,yz], 4 groups per op
            GB = 4
            for b in range(G // GB):
                gsl = slice(b * GB, (b + 1) * GB)
                S = s_pool.tile([P, GB * NCELL], BF16, tag="S")
                eng = nc.gpsimd if b >= 2 else nc.vector
                eng.tensor_tensor(
                    out=S[:].rearrange("p (g x c) -> p g x c", x=RES, c=64),
                    in0=Wv[:, gsl, 0, :].unsqueeze(3).to_broadcast([P, GB, RES, 64]),
                    in1=T1[:].rearrange("p (g c) -> p g c", c=64)[:, gsl]
                        .unsqueeze(2).to_broadcast([P, GB, RES, 64]),
                    op=OP.mult)
                for j in range(GB):
                    g = b * GB + j
                    nc.tensor.matmul(
                        out=(img_a if g % 2 == 0 else img_b)[:],
                        lhsT=feat_bf[:, CH * g:CH * (g + 1)],
                        rhs=S[:, NCELL * j:NCELL * (j + 1)],
                        start=(s == 0 and g < 2),
                        stop=(s == n_slabs - 1 and g >= G - 2),
                    )

        # ---------------- Phase 2: AllReduce ----------------
        img_as = lat_pool.tile([CH, NCELL], F32, tag="imgas")
        nc.scalar.copy(img_as[:], img_a[:])
        img_sb = lat_pool.tile([CH, NCELL], F32)
        nc.vector.tensor_tensor(out=img_sb[:], in0=img_as[:], in1=img_b[:],
                                op=OP.add)
        nc.sync.dma_start(ar_in[:], img_sb[:])
        nc.gpsimd.collective_compute(
            "AllReduce", OP.add,
            replica_groups=[list(range(NCORES))],
            ins=[ar_in[:]], outs=[ar_out[:]],
        )
        imgT = lat_pool.tile([CH, NCELL], F32)
        nc.sync.dma_start(imgT[:], ar_out[:])

        # ---------------- Phase 3: convs + gather table ----------------
        pad1 = lat_pool.tile([CH, 1000], BF16)
        for dst, src in _pad3_views(pad1[:], imgT[:]):
            nc.scalar.activation(dst, src, AF.Copy)
        enc_ps = psum_conv.tile([CH, NCELL], F32, tag="conv")
        _conv_layer(nc, enc_ps[:], pad1[:], wenc_b[:])
        r_bf = lat_pool.tile([CH, NCELL], BF16)
        nc.scalar.copy(r_bf[:], enc_ps[:])

        pad2 = lat_pool.tile([CH, 1000], BF16)
        for dst, src in _pad3_views(pad2[:], r_bf[:]):
            nc.scalar.activation(dst, src, AF.Copy)
        inn_ps = psum_conv.tile([CH, NCELL], F32, tag="conv")
        _conv_layer(nc, inn_ps[:], pad2[:], winner_b[:])
        sil_bf = lat_pool.tile([CH, NCELL], BF16)
        nc.scalar.activation(sil_bf[:], inn_ps[:], AF.Silu)
        h_bf = lat_pool.tile([CH, NCELL], BF16)
        nc.vector.tensor_tensor(out=h_bf[:], in0=r_bf[:], in1=sil_bf[:], op=OP.add)

        pad3 = lat_pool.tile([CH, 1000], BF16)
        for dst, src in _pad3_views(pad3[:], h_bf[:]):
            nc.scalar.activation(dst, src, AF.Copy)
        dec_ps = psum_conv.tile([CH, NCELL], F32, tag="conv")
        _conv_layer(nc, dec_ps[:], pad3[:], wdec_b[:])
        L_bf = lat_pool.tile([CH, NCELL], BF16)
        nc.scalar.copy(L_bf[:], dec_ps[:])

        # box filter: T[b] = sum_{d in {0,1}^3} L[(b+d) % 8]
        tpad = lat_pool.tile([CH, 729], BF16)
        for dst, src in _pad3_views(tpad[:], L_bf[:], pad_lo=0, pad_hi=1):
            nc.scalar.activation(dst, src, AF.Copy)
        tv = tpad[:].rearrange("p (x y z) -> p x y z", y=9, z=9)
        TT = lat_pool.tile([CH, NCELL], BF16)
        first = True
        for dx in range(2):
            for dy in range(2):
                for dz in range(2):
                    v = tv[:, dx:dx + RES, dy:dy + RES, dz:dz + RES]
                    if first:
                        nc.vector.tensor_copy(
                            TT[:].rearrange("p (x y z) -> p x y z", y=RES, z=RES), v)
                        first = False
                    else:
                        nc.vector.tensor_tensor(
                            out=TT[:].rearrange("p (x y z) -> p x y z", y=RES, z=RES),
                            in0=TT[:].rearrange("p (x y z) -> p x y z", y=RES, z=RES),
                            in1=v, op=OP.add)

        # transpose TT [32, 512] -> T [512, 32]: f32 chunks to DRAM (indirect
        # path) + bf16 chunks in SBUF (PE path lhsT)
        tchunk_b = []
        for c in range(4):
            tr_ps = psum_tr.tile([P, CH], BF16, tag="tr")
            nc.tensor.transpose(out=tr_ps[:], in_=TT[:, P * c:P * (c + 1)],
                                identity=ident32b[:])
            tr_sb = lat_pool.tile([P, CH], F32, tag=f"trsb{c}")
            nc.vector.tensor_copy(tr_sb[:], tr_ps[:])
            nc.sync.dma_start(t_dram[P * c:P * (c + 1), :], tr_sb[:])
            tcb = lat_pool.tile([P, CH], BF16, tag=f"tcb{c}")
            nc.scalar.copy(tcb[:], tr_ps[:])
            tchunk_b.append(tcb)

        # ---------------- Phase 4: hybrid gather ----------------
        # gpsimd indirect path
        for s in sorted(gp_set):
            rows = slice(s * P, (s + 1) * P)
            gout = g_pool.tile([P, CH * G], F32, tag="goutg")
            for g in range(G):
                nc.gpsimd.indirect_dma_start(
                    out=gout[:, CH * g:CH * (g + 1)],
                    out_offset=None,
                    in_=t_dram[:],
                    in_offset=bass.IndirectOffsetOnAxis(
                        ap=bflat_i[s][:, g:g + 1], axis=0),
                )
            nc.sync.dma_start(out_ext[rows, :], gout[:])

        # PE one-hot path (quarter-slab = 512 points per PSUM accumulation)
        QRT = SLAB // 4
        for s in range(n_slabs):
            if s in gp_set:
                continue
            rows = slice(s * P, (s + 1) * P)
            brep = eq_pool.tile([P, SLAB], F16, tag="brep")
            nc.sync.dma_start(brep[:],
                              brow_dram[s:s + 1, :].to_broadcast([P, SLAB]))
            gout = g_pool.tile([P, CH * G], F32, tag="goutp")
            for h in range(4):
                outT_ps = psum_ot.tile([CH, QRT], F32, tag="ot")
                for c in range(4):
                    eqc = eq_pool.tile([P, QRT], BF16, tag="eqc")
                    if c == 3:
                        tmp = eq_pool.tile([P, QRT], F32, tag="eqtmp")
                        nc.scalar.activation(
                            tmp[:], brep[:, QRT * h:QRT * (h + 1)],
                            AF.Abs, bias=iota_neg[c][:], scale=1.0)
                        nc.scalar.activation(
                            eqc[:], tmp[:], AF.Relu, bias=bias_p1[:],
                            scale=-1.0)
                    else:
                        nc.vector.tensor_tensor(
                            out=eqc[:],
                            in0=brep[:, QRT * h:QRT * (h + 1)],
                            in1=iota_cell[c][:].to_broadcast([P, QRT]),
                            op=OP.is_equal)
                    nc.tensor.matmul(
                        out=outT_ps[:],
                        lhsT=tchunk_b[c][:],
                        rhs=eqc[:],
                        start=(c == 0),
                        stop=(c == 3),
                    )
                outT_sb = ot_pool.tile([CH, QRT], BF16, tag="otsb")
                nc.scalar.copy(outT_sb[:], outT_ps[:])
                for q in range(4):
                    g = 4 * h + q
                    tr_ps = psum_tr.tile([P, CH], BF16, tag="tr")
                    nc.tensor.transpose(out=tr_ps[:],
                                        in_=outT_sb[:, P * q:P * (q + 1)],
                                        identity=ident32b[:])
                    ceng = nc.scalar if q % 2 == 0 else nc.vector
                    if q % 2 == 0:
                        nc.scalar.copy(gout[:, CH * g:CH * (g + 1)], tr_ps[:])
                    else:
                        nc.vector.tensor_copy(gout[:, CH * g:CH * (g + 1)],
                                              tr_ps[:])
            nc.scalar.dma_start(out_ext[rows, :], gout[:])

    _legalize_sync_waits(nc)
    return nc


_BUILT = {}
LAST_RESULT = None


def kernel(pos, input, encoder_kernel, inner_kernel, decoder_kernel):
    n = pos.shape[0]
    npc = -(-n // (NCORES * SLAB)) * SLAB        # points per core, slab multiple
    n_slabs = npc // SLAB
    npad = npc * NCORES

    pos_p = np.zeros((npad, 3), np.float32)
    pos_p[:n] = pos
    pos_p[n:] = 0.5
    feat_p = np.zeros((npad, CH), np.float32)
    feat_p[:n] = input

    # conv kernels -> lhsT layout [cin, (offset, cout)]
    def wprep(k):
        k = np.asarray(k, np.float32)            # [3,3,3,cout,cin]
        return np.ascontiguousarray(
            k.transpose(4, 0, 1, 2, 3).reshape(CH, 27 * CH))

    wenc = wprep(encoder_kernel)
    winner = wprep(np.asarray(inner_kernel, np.float32)[0])
    wdec = wprep(decoder_kernel)

    if n_slabs not in _BUILT:
        _BUILT[n_slabs] = build_nc(n_slabs)
    nc = _BUILT[n_slabs]

    nrows = n_slabs * P
    in_maps = []
    for c in range(NCORES):
        sl = slice(c * npc, (c + 1) * npc)
        in_maps.append({
            "pos": pos_p[sl].reshape(nrows, 3 * G),
            "feat": feat_p[sl].reshape(nrows, CH * G),
            "wenc": wenc, "winner": winner, "wdec": wdec,
        })

    trace = os.environ.get("KERNEL_TRACE", "0") == "1"
    tkw = {}
    if trace:
        tkw["trace"] = True
        tc_env = os.environ.get("KERNEL_TRACE_CORES", "0")
        tkw["trace_cores"] = [int(x) for x in tc_env.split(",")]
        td = os.environ.get("KERNEL_TRACE_DIR")
        if td:
            tkw["tmpdir"] = td
    res = run_bass_kernel_spmd(nc, in_maps, list(range(NCORES)), **tkw)
    global LAST_RESULT
    LAST_RESULT = res
    out = np.concatenate(
        [res.results[c]["out"].reshape(npc, CH) for c in range(NCORES)], axis=0)
    return np.ascontiguousarray(out[:n])


if __name__ == "__main__":
    rng = np.random.default_rng(0)
    n = 4096
    pos = rng.random((n, 3), np.float32).astype(np.float32)
    feat = rng.standard_normal((n, CH)).astype(np.float32)
    ek = rng.standard_normal((3, 3, 3, CH, CH)).astype(np.float32)
    ik = rng.standard_normal((1, 3, 3, 3, CH, CH)).astype(np.float32)
    dk = rng.standard_normal((3, 3, 3, CH, CH)).astype(np.float32)
    out = kernel(pos, feat, ek, ik, dk)
    print("out", out.shape, out.dtype, float(np.abs(out).max()))


# revision 13
# speedup vs baseline: 1.0238x; 1.0238x over previous
"""Trainium2 Bass kernel for the trilinear scatter -> conv3d x3 -> gather module.

Algorithm (per core, data-parallel over points):
  Phase 1 (scatter): for each 2048-point slab, build the trilinear scatter
    matrix S [128, 512] per 128-point group via separable periodic hat
    functions (relu(||iota-8p|-4|-3), all on ScalarE), outer-product them in
    bf16 (DVE 2x mode), and accumulate img^T[32f, 512cells] with PE matmuls
    alternating two PSUM banks. Base cell ids stay in SBUF (int32) for the
    gpsimd gather path and go to DRAM as an fp16 row (transposed to
    point-major g*128+p order) for the PE gather path.
  Phase 2: AllReduce img^T across the 8 cores (64KB).
  Phase 3 (redundant on all cores): 3 periodic 3x3x3 convs in bf16 as 27
    PSUM-accumulated matmuls each over a wrap-padded [32, 10^3] lattice,
    with silu residual; box-filter -> gather table T[512, 32] (f32 in DRAM
    for the indirect path, bf16 chunks in SBUF for the PE path).
  Phase 4 (hybrid gather): ~30% of slabs gather T rows via gpsimd indirect
    DMA; the rest run a PE one-hot path: broadcast the fp16 cell-id row to
    128 partitions (HWDGE), is_equal vs per-partition cell iota (bf16
    one-hot), 4 matmuls vs T^T chunks into PSUM, PE-transpose back to
    point-major, assemble on ScalarE.
"""
import os
import sys

for _p in ("/root/.axon_site/_ro/trn_rl_repo", "/opt/trn_rl_repo"):
    if _p not in sys.path and os.path.isdir(_p):
        sys.path.append(_p)

import numpy as np
from contextlib import ExitStack

import bass_rust
import concourse.bass as bass
import concourse.tile as tile
from concourse import mybir
from concourse.bass_utils import run_bass_kernel_spmd
from concourse.masks import make_identity

F32 = mybir.dt.float32
BF16 = mybir.dt.bfloat16
F16 = mybir.dt.float16
I32 = mybir.dt.int32
AF = mybir.ActivationFunctionType
OP = mybir.AluOpType

NCORES = 8
P = 128
G = 16                  # point-groups per slab (points per partition)
SLAB = P * G            # 2048 points per slab
RES = 8
NCELL = RES ** 3        # 512
CH = 32                 # cin = chid = cout = 32


def _legalize_sync_waits(nc, max_waits=1):
    """The walrus codegen on this path rejects instructions with more than
    ~1 extra sync wait; split excess waits onto preceding same-engine NoOps."""
    ctr = 0
    for f in nc.m.functions:
        for bb in f.blocks:
            insts = list(bb.instructions)
            new_insts = []
            changed = False
            for inst in insts:
                si = inst.sync_info
                if si is not None and len(si.on_wait) > max_waits:
                    waits = list(si.on_wait)
                    excess, keep = waits[:-max_waits], waits[-max_waits:]
                    for c0 in range(0, len(excess), max_waits):
                        chunk = excess[c0:c0 + max_waits]
                        nop = mybir.InstNoOp(name=f"lgw-{ctr}", ins=[], outs=[])
                        ctr += 1
                        nop.engine = inst.engine
                        nop.sync_info = bass_rust.SyncInfo(on_wait=chunk, on_update=[])
                        new_insts.append(nop)
                    si.on_wait = keep
                    changed = True
                new_insts.append(inst)
            if changed:
                bb.instructions = new_insts
    return nc


def _pad3_views(dst_pad, src, n=RES, pad_lo=1, pad_hi=1):
    """Yield (dst_view, src_view) block pairs to fill a wrap-padded lattice.

    dst_pad: AP [CH, (n+pad_lo+pad_hi)^3] ; src: AP [CH, n^3].
    """
    m = n + pad_lo + pad_hi
    dv = dst_pad.rearrange("p (x y z) -> p x y z", y=m, z=m)
    sv = src.rearrange("p (x y z) -> p x y z", y=n, z=n)
    segs = []
    if pad_lo:
        segs.append((0, n - pad_lo, pad_lo))
    segs.append((pad_lo, 0, n))
    if pad_hi:
        segs.append((pad_lo + n, 0, pad_hi))
    for dx, sx, lx in segs:
        for dy, sy, ly in segs:
            for dz, sz, lz in segs:
                yield (
                    dv[:, dx:dx + lx, dy:dy + ly, dz:dz + lz],
                    sv[:, sx:sx + lx, sy:sy + ly, sz:sz + lz],
                )


def _conv_layer(nc, psum_out, pad_t, w_t, m=10):
    """27 PSUM-accumulated matmuls: out[32co, 512] += K_o^T @ pad[32ci, shifted]."""
    pv = pad_t.rearrange("p (x y z) -> p x y z", y=m, z=m)
    o = 0
    for kx in range(3):
        for ky in range(3):
            for kz in range(3):
                rhs = pv[:, kx:kx + RES, ky:ky + RES, kz:kz + RES]
                nc.tensor.matmul(
                    out=psum_out,
                    lhsT=w_t[:, 32 * o:32 * (o + 1)],
                    rhs=rhs,
                    start=(o == 0),
                    stop=(o == 26),
                )
                o += 1


def build_nc(n_slabs):
    nrows = n_slabs * P          # rows in slab-layout [nrows, G*d]
    n_gp = max(0, min(n_slabs, round(n_slabs * 8 / 31)))  # gpsimd-path slabs
    gp_set = set(range(n_slabs - n_gp, n_slabs))
    nc = bass.Bass("TRN2", target_bir_lowering=False, debug=False,
                   num_devices=NCORES)

    pos_ext = nc.dram_tensor("pos", [nrows, 3 * G], F32, kind="ExternalInput").ap()
    feat_ext = nc.dram_tensor("feat", [nrows, CH * G], F32, kind="ExternalInput").ap()
    wenc_ext = nc.dram_tensor("wenc", [CH, 27 * CH], F32, kind="ExternalInput").ap()
    winner_ext = nc.dram_tensor("winner", [CH, 27 * CH], F32, kind="ExternalInput").ap()
    wdec_ext = nc.dram_tensor("wdec", [CH, 27 * CH], F32, kind="ExternalInput").ap()
    out_ext = nc.dram_tensor("out", [nrows, CH * G], F32, kind="ExternalOutput").ap()

    t_dram = nc.dram_tensor("t_table", [NCELL, CH], F32).ap()
    brow_dram = nc.dram_tensor("brow", [max(n_slabs, 1), SLAB], F16).ap()
    ar_in = nc.dram_tensor("ar_in", [CH, NCELL], F32).ap()
    ar_out = nc.dram_tensor("ar_out", [CH, NCELL], F32).ap()

    with tile.TileContext(nc) as tc, ExitStack() as ctx:
        const_pool = ctx.enter_context(tc.tile_pool(name="const", bufs=1))
        in_pool = ctx.enter_context(tc.tile_pool(name="in", bufs=3))
        work_pool = ctx.enter_context(tc.tile_pool(name="work", bufs=3))
        s_pool = ctx.enter_context(tc.tile_pool(name="s", bufs=4))
        lat_pool = ctx.enter_context(tc.tile_pool(name="lat", bufs=1))
        g_pool = ctx.enter_context(tc.tile_pool(name="g", bufs=4))
        bi_pool = ctx.enter_context(tc.tile_pool(name="bi", bufs=1))
        eq_pool = ctx.enter_context(tc.tile_pool(name="eq", bufs=3))
        ot_pool = ctx.enter_context(tc.tile_pool(name="ot", bufs=4))
        psum_img = ctx.enter_context(tc.tile_pool(name="pimg", bufs=1, space="PSUM"))
        psum_conv = ctx.enter_context(tc.tile_pool(name="pconv", bufs=1, space="PSUM"))
        psum_tr = ctx.enter_context(tc.tile_pool(name="ptr", bufs=2, space="PSUM"))
        psum_bft = ctx.enter_context(tc.tile_pool(name="pbft", bufs=1, space="PSUM"))
        psum_ot = ctx.enter_context(tc.tile_pool(name="pot", bufs=2, space="PSUM"))

        # constants
        iota_i = const_pool.tile([P, RES], I32)
        nc.gpsimd.iota(iota_i[:], pattern=[[1, RES]], base=0, channel_multiplier=0)
        iota_f = const_pool.tile([P, RES], F32)
        nc.scalar.copy(iota_f[:], iota_i[:])
        bias_m4 = const_pool.tile([P, 1], F32)
        nc.gpsimd.memset(bias_m4[:], -4.0)
        bias_m3 = const_pool.tile([P, 1], F32)
        nc.gpsimd.memset(bias_m3[:], -3.0)
        ident32b = const_pool.tile([CH, CH], BF16)
        make_identity(nc, ident32b[:])
        ident128h = const_pool.tile([P, P], F16)
        make_identity(nc, ident128h[:])
        # per-partition cell ids for the 4 one-hot chunks: c*128 + p (fp16)
        iota_cell = []
        for c in range(4):
            ii = const_pool.tile([P, 1], I32, tag=f"ioc{c}")
            nc.gpsimd.iota(ii[:], pattern=[[0, 1]], base=c * P,
                           channel_multiplier=1)
            ih = const_pool.tile([P, 1], F16, tag=f"ioh{c}")
            nc.scalar.copy(ih[:], ii[:])
            iota_cell.append(ih)

        iota_neg = []
        for c in range(4):
            iv = const_pool.tile([P, 1], F32, tag=f"ion{c}", name=f"ion{c}")
            nc.scalar.activation(iv[:], iota_cell[c][:], AF.Copy, scale=-1.0)
            iota_neg.append(iv)
        bias_p1 = const_pool.tile([P, 1], F32)
        nc.gpsimd.memset(bias_p1[:], 1.0)

        # weights
        wenc = const_pool.tile([CH, 27 * CH], F32)
        nc.sync.dma_start(wenc[:], wenc_ext[:])
        winner = const_pool.tile([CH, 27 * CH], F32)
        nc.sync.dma_start(winner[:], winner_ext[:])
        wdec = const_pool.tile([CH, 27 * CH], F32)
        nc.sync.dma_start(wdec[:], wdec_ext[:])
        wenc_b = const_pool.tile([CH, 27 * CH], BF16)
        nc.vector.tensor_copy(wenc_b[:], wenc[:])
        winner_b = const_pool.tile([CH, 27 * CH], BF16)
        nc.vector.tensor_copy(winner_b[:], winner[:])
        wdec_b = const_pool.tile([CH, 27 * CH], BF16)
        nc.vector.tensor_copy(wdec_b[:], wdec[:])

        img_a = psum_img.tile([CH, NCELL], F32, tag="a")
        img_b = psum_img.tile([CH, NCELL], F32, tag="b")

        bflat_i = {}
        # ---------------- Phase 1: scatter ----------------
        for s in range(n_slabs):
            rows = slice(s * P, (s + 1) * P)
            pos_t = in_pool.tile([P, 3 * G], F32, tag="pos")
            nc.sync.dma_start(pos_t[:], pos_ext[rows, :])
            feat_t = in_pool.tile([P, CH * G], F32, tag="feat")
            nc.scalar.dma_start(feat_t[:], feat_ext[rows, :])

            # sc = pos * 8
            sc = work_pool.tile([P, 3 * G], F32, tag="sc")
            nc.vector.tensor_scalar(out=sc[:], in0=pos_t[:], scalar1=float(RES),
                                    scalar2=None, op0=OP.mult)
            # exact floor: br = round(sc); bf = float(br); bff = bf - (bf > sc)
            br = work_pool.tile([P, 3 * G], I32, tag="br")
            nc.vector.tensor_copy(br[:], sc[:])
            bf = work_pool.tile([P, 3 * G], F32, tag="bf")
            nc.scalar.copy(bf[:], br[:])
            gt = work_pool.tile([P, 3 * G], F32, tag="gt")
            nc.vector.tensor_tensor(out=gt[:], in0=bf[:], in1=sc[:], op=OP.is_gt)
            bff = work_pool.tile([P, 3 * G], F32, tag="bff")
            nc.vector.tensor_tensor(out=bff[:], in0=bf[:], in1=gt[:], op=OP.subtract)

            # b_flat = bx*64 + by*8 + bz (exact in f32)
            bff3 = bff[:].rearrange("p (g d) -> p g d", d=3)
            t64 = work_pool.tile([P, G], F32, tag="t64")
            nc.vector.tensor_scalar(out=t64[:], in0=bff3[:, :, 0], scalar1=64.0,
                                    scalar2=None, op0=OP.mult)
            t8 = work_pool.tile([P, G], F32, tag="t8")
            nc.vector.tensor_scalar(out=t8[:], in0=bff3[:, :, 1], scalar1=8.0,
                                    scalar2=None, op0=OP.mult)
            bflat_f = work_pool.tile([P, G], F32, tag="bflat")
            nc.vector.tensor_tensor(out=bflat_f[:], in0=t64[:], in1=t8[:], op=OP.add)
            nc.vector.tensor_tensor(out=bflat_f[:], in0=bflat_f[:], in1=bff3[:, :, 2],
                                    op=OP.add)

            if s in gp_set:
                bfi = bi_pool.tile([P, G], I32, tag=f"bfi{s}")
                nc.vector.tensor_copy(bfi[:], bflat_f[:])
                bflat_i[s] = bfi
            else:
                # fp16 ids, transposed to point-major j = g*128+p, to DRAM
                bflat_h = work_pool.tile([P, G], F16, tag="bfh")
                nc.vector.tensor_copy(bflat_h[:], bflat_f[:])
                bfT_ps = psum_bft.tile([G, P], F16, tag="bfT")
                nc.tensor.transpose(out=bfT_ps[:], in_=bflat_h[:],
                                    identity=ident128h[:])
                bfT = work_pool.tile([G, P], F16, tag="bfTs")
                nc.scalar.copy(bfT[:], bfT_ps[:])
                nc.sync.dma_start(
                    brow_dram[s:s + 1, :].rearrange("r (p c) -> (r p) c", c=P),
                    bfT[:])

            # periodic hat weights, all dims at once: W[p, (g, d, 8)] bf16
            # ring-dist hat: W = relu(||iota - sc| - 4| - 3)
            E = work_pool.tile([P, G * 3 * RES], F32, tag="E")
            nc.vector.tensor_tensor(
                out=E[:].rearrange("p (g d e) -> p g d e", d=3, e=RES),
                in0=iota_f[:].unsqueeze(1).unsqueeze(1)
                    .to_broadcast([P, G, 3, RES]),
                in1=sc[:].rearrange("p (g d) -> p g d", d=3).unsqueeze(3)
                    .to_broadcast([P, G, 3, RES]),
                op=OP.subtract)
            A = work_pool.tile([P, G * 3 * RES], F32, tag="A")
            nc.scalar.activation(A[:], E[:], AF.Abs)
            B2 = work_pool.tile([P, G * 3 * RES], F32, tag="B2")
            nc.scalar.activation(B2[:], A[:], AF.Abs, bias=bias_m4[:], scale=1.0)
            W = work_pool.tile([P, G * 3 * RES], BF16, tag="W")
            nc.scalar.activation(W[:], B2[:], AF.Relu, bias=bias_m3[:], scale=1.0)
            Wv = W[:].rearrange("p (g d e) -> p g d e", d=3, e=RES)

            # T1[p, (g, y, z)] = WY[g,y] * WZ[g,z]  (bf16, 2x mode)
            T1 = work_pool.tile([P, G * 64], BF16, tag="T1")
            nc.vector.tensor_tensor(
                out=T1[:].rearrange("p (g y z) -> p g y z", y=RES, z=RES),
                in0=Wv[:, :, 1, :].unsqueeze(3).to_broadcast([P, G, RES, RES]),
                in1=Wv[:, :, 2, :].unsqueeze(2).to_broadcast([P, G, RES, RES]),
                op=OP.mult)

            # feat -> bf16 for PE
            feat_bf = work_pool.tile([P, CH * G], BF16, tag="featbf")
            nc.scalar.copy(feat_bf[:], feat_t[:])

            # S[p, (g, x, yz)] = WX[g,x] * T1[g,yz], 4 groups per op
            GB = 4
            for b in range(G // GB):
                gsl = slice(b * GB, (b + 1) * GB)
                S = s_pool.tile([P, GB * NCELL], BF16, tag="S")
                eng = nc.gpsimd if b >= 2 else nc.vector
                eng.tensor_tensor(
                    out=S[:].rearrange("p (g x c) -> p g x c", x=RES, c=64),
                    in0=Wv[:, gsl, 0, :].unsqueeze(3).to_broadcast([P, GB, RES, 64]),
                    in1=T1[:].rearrange("p (g c) -> p g c", c=64)[:, gsl]
                        .unsqueeze(2).to_broadcast([P, GB, RES, 64]),
                    op=OP.mult)
                for j in range(GB):
                    g = b * GB + j
                    nc.tensor.matmul(
                        out=(img_a if g % 2 == 0 else img_b)[:],
                        lhsT=feat_bf[:, CH * g:CH * (g + 1)],
                        rhs=S[:, NCELL * j:NCELL * (j + 1)],
                        start=(s == 0 and g < 2),
                        stop=(s == n_slabs - 1 and g >= G - 2),
                    )

        # ---------------- Phase 2: AllReduce ----------------
        img_as = lat_pool.tile([CH, NCELL], F32, tag="imgas")
        nc.scalar.copy(img_as[:], img_a[:])
        img_sb = lat_pool.tile([CH, NCELL], F32)
        nc.vector.tensor_tensor(out=img_sb[:], in0=img_as[:], in1=img_b[:],
                                op=OP.add)
        nc.sync.dma_start(ar_in[:], img_sb[:])
        nc.gpsimd.collective_compute(
            "AllReduce", OP.add,
            replica_groups=[list(range(NCORES))],
            ins=[ar_in[:]], outs=[ar_out[:]],
        )
        imgT = lat_pool.tile([CH, NCELL], F32)
        nc.sync.dma_start(imgT[:], ar_out[:])

        # ---------------- Phase 3: convs + gather table ----------------
        pad1 = lat_pool.tile([CH, 1000], BF16)
        for dst, src in _pad3_views(pad1[:], imgT[:]):
            nc.scalar.activation(dst, src, AF.Copy)
        enc_ps = psum_conv.tile([CH, NCELL], F32, tag="conv")
        _conv_layer(nc, enc_ps[:], pad1[:], wenc_b[:])
        r_bf = lat_pool.tile([CH, NCELL], BF16)
        nc.scalar.copy(r_bf[:], enc_ps[:])

        pad2 = lat_pool.tile([CH, 1000], BF16)
        for dst, src in _pad3_views(pad2[:], r_bf[:]):
            nc.scalar.activation(dst, src, AF.Copy)
        inn_ps = psum_conv.tile([CH, NCELL], F32, tag="conv")
        _conv_layer(nc, inn_ps[:], pad2[:], winner_b[:])
        sil_bf = lat_pool.tile([CH, NCELL], BF16)
        nc.scalar.activation(sil_bf[:], inn_ps[:], AF.Silu)
        h_bf = lat_pool.tile([CH, NCELL], BF16)
        nc.vector.tensor_tensor(out=h_bf[:], in0=r_bf[:], in1=sil_bf[:], op=OP.add)

        pad3 = lat_pool.tile([CH, 1000], BF16)
        for dst, src in _pad3_views(pad3[:], h_bf[:]):
            nc.scalar.activation(dst, src, AF.Copy)
        dec_ps = psum_conv.tile([CH, NCELL], F32, tag="conv")
        _conv_layer(nc, dec_ps[:], pad3[:], wdec_b[:])
        L_bf = lat_pool.tile([CH, NCELL], BF16)
        nc.scalar.copy(L_bf[:], dec_ps[:])

        # box filter: T[b] = sum_{d in {0,1}^3} L[(b+d) % 8]
        tpad = lat_pool.tile([CH, 729], BF16)
        for dst, src in _pad3_views(tpad[:], L_bf[:], pad_lo=0, pad_hi=1):
            nc.scalar.activation(dst, src, AF.Copy)
        tv = tpad[:].rearrange("p (x y z) -> p x y z", y=9, z=9)
        TT = lat_pool.tile([CH, NCELL], BF16)
        first = True
        for dx in range(2):
            for dy in range(2):
                for dz in range(2):
                    v = tv[:, dx:dx + RES, dy:dy + RES, dz:dz + RES]
                    if first:
                        nc.vector.tensor_copy(
                            TT[:].rearrange("p (x y z) -> p x y z", y=RES, z=RES), v)
                        first = False
                    else:
                        nc.vector.tensor_tensor(
                            out=TT[:].rearrange("p (x y z) -> p x y z", y=RES, z=RES),
                            in0=TT[:].rearrange("p (x y z) -> p x y z", y=RES, z=RES),
                            in1=v, op=OP.add)

        # transpose TT [32, 512] -> T [512, 32]: f32 chunks to DRAM (indirect
        # path) + bf16 chunks in SBUF (PE path lhsT)
        tchunk_b = []
        for c in range(4):
            tr_ps = psum_tr.tile([P, CH], BF16, tag="tr")
            nc.tensor.transpose(out=tr_ps[:], in_=TT[:, P * c:P * (c + 1)],
                                identity=ident32b[:])
            tr_sb = lat_pool.tile([P, CH], F32, tag=f"trsb{c}")
            nc.vector.tensor_copy(tr_sb[:], tr_ps[:])
            nc.sync.dma_start(t_dram[P * c:P * (c + 1), :], tr_sb[:])
            tcb = lat_pool.tile([P, CH], BF16, tag=f"tcb{c}")
            nc.scalar.copy(tcb[:], tr_ps[:])
            tchunk_b.append(tcb)

        # ---------------- Phase 4: hybrid gather ----------------
        # gpsimd indirect path
        for s in sorted(gp_set):
            rows = slice(s * P, (s + 1) * P)
            gout = g_pool.tile([P, CH * G], F32, tag="goutg")
            for g in range(G):
                nc.gpsimd.indirect_dma_start(
                    out=gout[:, CH * g:CH * (g + 1)],
                    out_offset=None,
                    in_=t_dram[:],
                    in_offset=bass.IndirectOffsetOnAxis(
                        ap=bflat_i[s][:, g:g + 1], axis=0),
                )
            nc.sync.dma_start(out_ext[rows, :], gout[:])

        # PE one-hot path (quarter-slab = 512 points per PSUM accumulation)
        QRT = SLAB // 4
        for s in range(n_slabs):
            if s in gp_set:
                continue
            rows = slice(s * P, (s + 1) * P)
            brep = eq_pool.tile([P, SLAB], F16, tag="brep")
            nc.sync.dma_start(brep[:],
                              brow_dram[s:s + 1, :].to_broadcast([P, SLAB]))
            gout = g_pool.tile([P, CH * G], F32, tag="goutp")
            for h in range(4):
                outT_ps = psum_ot.tile([CH, QRT], F32, tag="ot")
                for c in range(4):
                    eqc = eq_pool.tile([P, QRT], BF16, tag="eqc")
                    if c == 3:
                        tmp = eq_pool.tile([P, QRT], F32, tag="eqtmp")
                        nc.scalar.activation(
                            tmp[:], brep[:, QRT * h:QRT * (h + 1)],
                            AF.Abs, bias=iota_neg[c][:], scale=1.0)
                        nc.scalar.activation(
                            eqc[:], tmp[:], AF.Relu, bias=bias_p1[:],
                            scale=-1.0)
                    else:
                        nc.vector.tensor_tensor(
                            out=eqc[:],
                            in0=brep[:, QRT * h:QRT * (h + 1)],
                            in1=iota_cell[c][:].to_broadcast([P, QRT]),
                            op=OP.is_equal)
                    nc.tensor.matmul(
                        out=outT_ps[:],
                        lhsT=tchunk_b[c][:],
                        rhs=eqc[:],
                        start=(c == 0),
                        stop=(c == 3),
                    )
                outT_sb = ot_pool.tile([CH, QRT], BF16, tag="otsb")
                nc.scalar.copy(outT_sb[:], outT_ps[:])
                for q in range(4):
                    g = 4 * h + q
                    tr_ps = psum_tr.tile([P, CH], BF16, tag="tr")
                    nc.tensor.transpose(out=tr_ps[:],
                                        in_=outT_sb[:, P * q:P * (q + 1)],
                                        identity=ident32b[:])
                    nc.scalar.copy(gout[:, CH * g:CH * (g + 1)], tr_ps[:])
            nc.scalar.dma_start(out_ext[rows, :], gout[:])

    _legalize_sync_waits(nc)
    return nc


_BUILT = {}
LAST_RESULT = None


def kernel(pos, input, encoder_kernel, inner_kernel, decoder_kernel):
    n = pos.shape[0]
    npc = -(-n // (NCORES * SLAB)) * SLAB        # points per core, slab multiple
    n_slabs = npc // SLAB
    npad = npc * NCORES

    pos_p = np.zeros((npad, 3), np.float32)
    pos_p[:n] = pos
    pos_p[n:] = 0.5
    feat_p = np.zeros((npad, CH), np.float32)
    feat_p[:n] = input

    # conv kernels -> lhsT layout [cin, (offset, cout)]
    def wprep(k):
        k = np.asarray(k, np.float32)            # [3,3,3,cout,cin]
        return np.ascontiguousarray(
            k.transpose(4, 0, 1, 2, 3).reshape(CH, 27 * CH))

    wenc = wprep(encoder_kernel)
    winner = wprep(np.asarray(inner_kernel, np.float32)[0])
    wdec = wprep(decoder_kernel)

    if n_slabs not in _BUILT:
        _BUILT[n_slabs] = build_nc(n_slabs)
    nc = _BUILT[n_slabs]

    nrows = n_slabs * P
    in_maps = []
    for c in range(NCORES):
        sl = slice(c * npc, (c + 1) * npc)
        in_maps.append({
            "pos": pos_p[sl].reshape(nrows, 3 * G),
            "feat": feat_p[sl].reshape(nrows, CH * G),
            "wenc": wenc, "winner": winner, "wdec": wdec,
        })

    trace = os.environ.get("KERNEL_TRACE", "0") == "1"
    tkw = {}
    if trace:
        tkw["trace"] = True
        tc_env = os.environ.get("KERNEL_TRACE_CORES", "0")
        tkw["trace_cores"] = [int(x) for x in tc_env.split(",")]
        td = os.environ.get("KERNEL_TRACE_DIR")
        if td:
            tkw["tmpdir"] = td
    res = run_bass_kernel_spmd(nc, in_maps, list(range(NCORES)), **tkw)
    global LAST_RESULT
    LAST_RESULT = res
    out = np.concatenate(
        [res.results[c]["out"].reshape(npc, CH) for c in range(NCORES)], axis=0)
    return np.ascontiguousarray(out[:n])


if __name__ == "__main__":
    rng = np.random.default_rng(0)
    n = 4096
    pos = rng.random((n, 3), np.float32).astype(np.float32)
    feat = rng.standard_normal((n, CH)).astype(np.float32)
    ek = rng.standard_normal((3, 3, 3, CH, CH)).astype(np.float32)
    ik = rng.standard_normal((1, 3, 3, 3, CH, CH)).astype(np.float32)
    dk = rng.standard_normal((3, 3, 3, CH, CH)).astype(np.float32)
    out = kernel(pos, feat, ek, ik, dk)
    print("out", out.shape, out.dtype, float(np.abs(out).max()))
